# revision 19
# baseline (speedup 1.0000x reference)
"""ATACSplitPool Trainium2 kernel.

kernel(**inputs) takes the FULL inputs of the reference problem
(B=8, L=25000, D=639, PATCH=25, max_n_peaks=200) and returns the full
(8, 200, 800) float32 output, computed on 8 NeuronCores (one sample per
core, data-parallel over batch; small conv weights replicated).

Host side does only layout prep: dtype cast (fp16), padding/permutation
of x, weight transposes, and turning the ragged segment structure
(peak_split) into dense 0/1 pooling matrices + count reciprocals.
All reductions/compute over the data happen on device.

Device dataflow per core (own sample):
  - DVE fold trees over fp16 x in [128ch, 25, 500] tiles -> patch max
    (fp16) and patch sum (f32) per channel.
  - atac path replicated over the batch (atac is tiny), so BatchNorm1
    stats are local; conv1 via PE with K=3.
  - conv2 = 21 accumulating fp16 matmuls per output half (7 K-tiles x
    3 taps); BatchNorm2 stats go through a tiny AllReduce (a warm-up
    AllReduce at kernel start hides the ~70us first-collective ramp).
  - segment pooling = matmuls against host-built 0/1 matrices after
    PE transposes of the pooled data.
"""
import math
from contextlib import ExitStack

import numpy as np

import concourse.bass as bass
import concourse.tile as tile
from concourse import bacc, mybir
from concourse.bass_utils import run_bass_kernel_spmd
from concourse.masks import make_identity

dt = mybir.dt
F16 = dt.float16
F32 = dt.float32
AF = mybir.ActivationFunctionType
ALU = mybir.AluOpType

B, L, D = 8, 25000, 639
PATCH = 25
LP = 1000            # L // PATCH
MAXP = 200           # max_n_peaks
CJ = 161             # joint conv out channels
CH = 800             # conv2 in channels = D + CJ
NCORES = 8
DPAD = 640           # D padded to 5*128
NDT = 5              # D tiles of 128
TCH = 500            # fold chunk width in Lp positions
NJ = 2               # fold chunks per D tile (LP / TCH)
BN_EPS = 1e-5

KT = [128, 128, 128, 128, 127]      # x_pooled channel K tiles
K0 = [0, 128, 256, 384, 512, 639, 767]
KSZ = KT + [128, 33]
MT = [(0, 128), (128, 72)]          # chunk-row M tiles (200 rows)
AT = [(0, 128), (128, 33)]          # atac-channel tiles of 161
NT2 = [(0, 499), (499, 501)]        # conv2 t-halves (bank-sized, and the
                                    # first depends only on fold round 0)

_COMPILED = None


def _build(dbg=False, warm_cc=True):
    nc = bacc.Bacc("TRN2", target_bir_lowering=False, debug=False,
                   num_devices=NCORES)

    xq = nc.dram_tensor("xq", [NJ, NDT, 128, PATCH, TCH], F16,
                        kind="ExternalInput").ap()
    atc = nc.dram_tensor("atc", [125, 8 * 200], F32,
                         kind="ExternalInput").ap()
    w1 = nc.dram_tensor("w1", [3, CJ], F32, kind="ExternalInput").ap()
    w2 = nc.dram_tensor("w2", [128, 21 * CJ], F16,
                        kind="ExternalInput").ap()
    qsb = nc.dram_tensor("qsb", [125, 8 * MAXP], F16,
                         kind="ExternalInput").ap()
    inve = nc.dram_tensor("inve", [MAXP], F32, kind="ExternalInput").ap()
    invp = nc.dram_tensor("invp", [MAXP], F32, kind="ExternalInput").ap()
    out = nc.dram_tensor("out", [MAXP, CH], F32, kind="ExternalOutput").ap()

    cc_in = nc.dram_tensor("cc_in", [2, CJ], F32)
    cc_out = nc.dram_tensor("cc_out", [2, CJ], F32, addr_space="Shared")

    if dbg:
        d_zt = nc.dram_tensor("d_zt", [128, LP], F32,
                              kind="ExternalOutput").ap()
        d_g = nc.dram_tensor("d_g", [128, 2], F32,
                             kind="ExternalOutput").ap()
        d_h2T = nc.dram_tensor("d_h2T", [125, 8 * CJ], F16,
                               kind="ExternalOutput").ap()

    with tile.TileContext(nc) as tc, ExitStack() as ctx:
        keep = ctx.enter_context(tc.tile_pool(name="keep", bufs=1))
        wk = ctx.enter_context(tc.tile_pool(name="wk", bufs=2))
        pp = ctx.enter_context(tc.tile_pool(name="pp", bufs=2, space="PSUM"))

        # ---- warm-up collective on the (junk) stats buffers ----
        if warm_cc:
            nc.gpsimd.collective_compute(
                "AllReduce", ALU.add, replica_groups=[list(range(NCORES))],
                ins=[cc_in.ap()], outs=[cc_out.ap()])

        # ---- persistent small loads (SWDGE; keeps sync free for xq) ----
        w1t = keep.tile([3, CJ], F32)
        nc.gpsimd.dma_start(w1t[:], w1[:])
        w2t = keep.tile([128, 21 * CJ], F16)
        nc.gpsimd.dma_start(w2t[:], w2[:])
        qt = keep.tile([125, 8 * MAXP], F16)
        nc.gpsimd.dma_start(qt[:], qsb[:])
        atct = keep.tile([125, 8 * 200], F32)
        nc.gpsimd.dma_start(atct[:], atc[:])
        invet, invpt = [], []
        for (m0, msz) in MT:
            t_e = keep.tile([msz, 1], F32, name=f"inve{m0}")
            nc.gpsimd.dma_start(t_e[:], inve[m0:m0 + msz])
            invet.append(t_e)
            t_p = keep.tile([msz, 1], F32, name=f"invp{m0}")
            nc.gpsimd.dma_start(t_p[:], invp[m0:m0 + msz])
            invpt.append(t_p)
        idf = keep.tile([128, 128], F32)
        make_identity(nc, idf)

        # ---- persistent result buffers ----
        maxb = []
        for dtile in range(NDT):
            mb = keep.tile([128, LP + 2], F16, name=f"maxb{dtile}")
            nc.vector.memset(mb[:, 0:1], 0.0)
            nc.vector.memset(mb[:, LP + 1:LP + 2], 0.0)
            maxb.append(mb)
        S = [keep.tile([128, LP], F32, name=f"S{dtile}")
             for dtile in range(NDT)]

        # =====================================================
        # main fold: patch max + patch sum over x (fp16, 2x DVE mode)
        # j-major so conv2's first half can start after round 0
        # =====================================================
        iop = ctx.enter_context(tc.tile_pool(name="io", bufs=2))
        fwp = ctx.enter_context(tc.tile_pool(name="fw", bufs=1))
        for j in range(NJ):
            for dtile in range(NDT):
                xin = iop.tile([128, PATCH * TCH], F16, tag="xin")
                nc.sync.dma_start(
                    xin[:], xq[j, dtile].rearrange("p r t -> p (r t)"))
                x3 = xin[:].rearrange("p (r t) -> p r t", r=PATCH)
                m12 = fwp.tile([128, 12, TCH], F16, tag="m12")
                s12 = fwp.tile([128, 12, TCH], F16, tag="s12")
                nc.vector.tensor_max(m12[:], x3[:, 0:12, :], x3[:, 12:24, :])
                nc.vector.tensor_add(s12[:], x3[:, 0:12, :], x3[:, 12:24, :])
                m6 = fwp.tile([128, 6, TCH], F16, tag="m6")
                s6 = fwp.tile([128, 6, TCH], F16, tag="s6")
                nc.vector.tensor_max(m6[:], m12[:, 0:6, :], m12[:, 6:12, :])
                nc.vector.tensor_add(s6[:], s12[:, 0:6, :], s12[:, 6:12, :])
                m3 = fwp.tile([128, 3, TCH], F16, tag="m3")
                s3 = fwp.tile([128, 3, TCH], F16, tag="s3")
                nc.vector.tensor_max(m3[:], m6[:, 0:3, :], m6[:, 3:6, :])
                nc.vector.tensor_add(s3[:], s6[:, 0:3, :], s6[:, 3:6, :])
                m1 = fwp.tile([128, 1, TCH], F16, tag="m1")
                s1 = fwp.tile([128, 1, TCH], F16, tag="s1")
                nc.vector.tensor_max(m1[:], m3[:, 0:1, :], m3[:, 1:2, :])
                nc.vector.tensor_add(s1[:], s3[:, 0:1, :], s3[:, 1:2, :])
                m1b = fwp.tile([128, 1, TCH], F16, tag="m1b")
                s1b = fwp.tile([128, 1, TCH], F16, tag="s1b")
                nc.vector.tensor_max(m1b[:], m1[:], m3[:, 2:3, :])
                nc.vector.tensor_add(s1b[:], s1[:], s3[:, 2:3, :])
                mdst = maxb[dtile][:, 1 + j * TCH:1 + j * TCH + TCH]
                nc.vector.tensor_max(
                    mdst.rearrange("p (o t) -> p o t", o=1),
                    m1b[:], x3[:, 24:25, :])
                sdst = S[dtile][:, bass.ts(j, TCH)]
                nc.vector.tensor_add(
                    sdst.rearrange("p (o t) -> p o t", o=1),
                    s1b[:], x3[:, 24:25, :])

        # =====================================================
        # atac path (replicated over all 8 samples; own sample is in
        # slot 0 because the host rotates atc per core)
        # =====================================================
        apool = keep.tile([125, 64], F32)
        for b in range(8):
            nc.vector.reduce_max(
                apool[:, bass.ts(b, 8)],
                atct[:, bass.ts(b, 200)].rearrange("p (g r) -> p g r",
                                                   r=PATCH),
                axis=mybir.AxisListType.X)
        alog = keep.tile([125, 64], F32)
        nc.scalar.activation(alog[:], apool[:], AF.Ln, bias=1.0)

        # conv1 + BN1 stats for all samples; keep slot-0 pre-activations
        arow = keep.tile([3, LP + 2], F32)
        nc.gpsimd.memset(arow[:], 0.0)
        apre = []   # slot-0 conv1 pre-BN, zero-edged [asz, LP+2]
        for (a0, asz) in AT:
            t = keep.tile([asz, LP + 2], F32, name=f"apre{a0}")
            nc.vector.memset(t[:, 0:1], 0.0)
            nc.vector.memset(t[:, LP + 1:LP + 2], 0.0)
            apre.append(t)
        # s1p layout: cols k*16 + n*8 + b  (k: 0=sum 1=sumsq)
        s1p = [keep.tile([asz, 32], F32, name=f"s1p{a0}")
               for (a0, asz) in AT]
        for b in range(8):
            # arow rows: [dk, t] = A[t + dk - 1] with zero edges
            nc.sync.dma_start(arow[0:1, 1:LP + 1], alog[:, bass.ts(b, 8)])
            nc.sync.dma_start(arow[1:2, 0:LP], alog[:, bass.ts(b, 8)])
            # row 2 = A shifted left; trailing zero from initial memset
            nc.sync.dma_start(arow[2:3, 0:7], alog[0:1, b * 8 + 1:b * 8 + 8])
            nc.sync.dma_start(arow[2:3, 7:LP - 1],
                              alog[1:125, bass.ts(b, 8)])
            for mi, (a0, asz) in enumerate(AT):
                for n in range(2):
                    za = pp.tile([asz, 512], F32, tag="pz")
                    nc.tensor.matmul(
                        za[:, 0:TCH], w1t[:, a0:a0 + asz],
                        arow[:, n * TCH:n * TCH + TCH],
                        start=True, stop=True)
                    if b == 0:
                        cdst = apre[mi][:, 1 + n * TCH:1 + (n + 1) * TCH]
                    else:
                        scr = wk.tile([asz, TCH], F16, tag=f"scr{mi}")
                        cdst = scr[:]
                    nc.scalar.activation(
                        cdst, za[:, 0:TCH], AF.Copy,
                        accum_out=s1p[mi][:, n * 8 + b:n * 8 + b + 1])
                    scr2 = wk.tile([asz, TCH], F16, tag=f"scr{mi}")
                    nc.scalar.activation(
                        scr2[:], za[:, 0:TCH], AF.Square,
                        accum_out=s1p[mi][:, 16 + n * 8 + b:17 + n * 8 + b])

        # BN1 coefficients (full-batch stats, local because replicated)
        def bn_coeffs(tag, asz, sums, count):
            mean = keep.tile([asz, 1], F32, name=f"m{tag}")
            nc.vector.tensor_scalar_mul(mean[:], sums[:, 0:1], 1.0 / count)
            var = keep.tile([asz, 1], F32, name=f"v{tag}")
            nc.vector.tensor_mul(var[:], mean[:], mean[:])
            msq = keep.tile([asz, 1], F32, name=f"q{tag}")
            nc.vector.tensor_scalar_mul(msq[:], sums[:, 1:2], 1.0 / count)
            nc.vector.tensor_sub(var[:], msq[:], var[:])
            nc.vector.tensor_scalar_add(var[:], var[:], BN_EPS)
            sig = keep.tile([asz, 1], F32, name=f"s{tag}")
            nc.scalar.activation(sig[:], var[:], AF.Sqrt, bias=0.0)
            isig = keep.tile([asz, 1], F32, name=f"i{tag}")
            nc.vector.reciprocal(isig[:], sig[:])
            nbias = keep.tile([asz, 1], F32, name=f"n{tag}")
            nc.vector.tensor_mul(nbias[:], mean[:], isig[:])
            nc.vector.tensor_scalar_mul(nbias[:], nbias[:], -1.0)
            return isig, nbias

        anrm = []
        for mi, (a0, asz) in enumerate(AT):
            red = keep.tile([asz, 2], F32, name=f"red1{a0}")
            nc.vector.reduce_sum(
                red[:], s1p[mi][:].rearrange("p (k b) -> p k b", b=16),
                axis=mybir.AxisListType.X)
            isig, nbias = bn_coeffs(f"1{a0}", asz, red, 8000.0)
            t = keep.tile([asz, LP + 2], F16, name=f"anrm{a0}")
            nc.vector.memset(t[:, 0:1], 0.0)
            nc.vector.memset(t[:, LP + 1:LP + 2], 0.0)
            nc.scalar.activation(t[:, 1:LP + 1], apre[mi][:, 1:LP + 1],
                                 AF.Relu, bias=nbias[:], scale=isig[:])
            anrm.append(t)

        # =====================================================
        # conv2 (own sample): z = W2 * [x_pooled; a], 3 taps.
        # t-half A only needs fold round 0; half B needs round 1.
        # =====================================================
        ktiles = [maxb[0], maxb[1], maxb[2], maxb[3], maxb[4],
                  anrm[0], anrm[1]]
        zs, s2p = [], []
        for mi, (a0, asz) in enumerate(AT):
            zt = keep.tile([asz, LP], F32, name=f"zt{mi}")
            spn = keep.tile([asz, 4], F32, name=f"s2pn{mi}")
            for n, (n0, nsz) in enumerate(NT2):
                zp = pp.tile([asz, 512], F32, tag="pz")
                i = 0
                for dk in range(3):
                    for kt in range(7):
                        ksz = KSZ[kt]
                        slot = dk * 7 + kt
                        nc.tensor.matmul(
                            zp[:, 0:nsz],
                            w2t[0:ksz, slot * CJ + a0:slot * CJ + a0 + asz],
                            ktiles[kt][0:ksz, dk + n0:dk + n0 + nsz],
                            start=(i == 0), stop=(i == 20))
                        i += 1
                nc.scalar.activation(zt[:, n0:n0 + nsz], zp[:, 0:nsz],
                                     AF.Copy, accum_out=spn[:, n:n + 1])
                zsq = wk.tile([asz, 512], F16, tag=f"scr{mi}")
                nc.scalar.activation(zsq[:, 0:nsz], zp[:, 0:nsz], AF.Square,
                                     accum_out=spn[:, 2 + n:3 + n])
            sp = keep.tile([asz, 2], F32, name=f"s2p{mi}")
            nc.vector.tensor_add(
                sp[:].rearrange("p (k o) -> p k o", o=1),
                spn[:].rearrange("p (k n) -> p k n", n=2)[:, :, 0:1],
                spn[:].rearrange("p (k n) -> p k n", n=2)[:, :, 1:2])
            zs.append(zt)
            s2p.append(sp)

        # ship stats through the AllReduce
        nc.sync.dma_start(cc_in[0:1, 0:128], s2p[0][:, 0:1])
        nc.sync.dma_start(cc_in[0:1, 128:161], s2p[1][:, 0:1])
        nc.sync.dma_start(cc_in[1:2, 0:128], s2p[0][:, 1:2])
        nc.sync.dma_start(cc_in[1:2, 128:161], s2p[1][:, 1:2])
        nc.gpsimd.collective_compute(
            "AllReduce", ALU.add, replica_groups=[list(range(NCORES))],
            ins=[cc_in.ap()], outs=[cc_out.ap()])

        # =====================================================
        # x_region: transpose S, pool with Q, scale by 1/len
        # (independent of conv2/collective; overlaps fold round 1)
        # =====================================================
        sT = keep.tile([125, 8 * DPAD], F16)
        for dtile in range(NDT):
            dsz = KT[dtile]
            for s in range(8):
                tp2 = pp.tile([125, 128], F32, tag="tp")
                nc.tensor.transpose(tp2[0:125, 0:dsz],
                                    S[dtile][0:dsz, bass.ts(s, 125)],
                                    idf[0:dsz, 0:dsz])
                nc.scalar.copy(
                    sT[:, s * DPAD + dtile * 128:
                       s * DPAD + dtile * 128 + dsz],
                    tp2[0:125, 0:dsz])

        NT = [(0, 512), (512, 127)]
        for mi, (m0, msz) in enumerate(MT):
            for (n0, nsz) in NT:
                xr = pp.tile([msz, 512], F32, tag="xr", bufs=1)
                for s in range(8):
                    nc.tensor.matmul(
                        xr[:, 0:nsz],
                        qt[:, s * MAXP + m0:s * MAXP + m0 + msz],
                        sT[:, s * DPAD + n0:s * DPAD + n0 + nsz],
                        start=(s == 0), stop=(s == 7))
                xro = wk.tile([msz, 512], F32, tag="xro")
                nc.scalar.activation(xro[:, 0:nsz], xr[:, 0:nsz], AF.Copy,
                                     scale=invet[mi][:])
                nc.sync.dma_start(out[m0:m0 + msz, n0:n0 + nsz],
                                  xro[:, 0:nsz])

        # =====================================================
        # BN2 apply + relu -> h2, transpose, pool with Q, log2(mean+1)
        # =====================================================
        gs = []
        for mi, (a0, asz) in enumerate(AT):
            gsum = keep.tile([asz, 2], F32, name=f"g2{a0}")
            nc.sync.dma_start(gsum[:, 0:1], cc_out[0:1, a0:a0 + asz])
            nc.sync.dma_start(gsum[:, 1:2], cc_out[1:2, a0:a0 + asz])
            gs.append(gsum)

        h2T = keep.tile([125, 8 * CJ], F16)
        for mi, (a0, asz) in enumerate(AT):
            isig, nbias = bn_coeffs(f"2{a0}", asz, gs[mi], 8000.0)
            h2 = keep.tile([asz, LP], F32, name=f"h2{mi}")
            nc.scalar.activation(h2[:], zs[mi][:], AF.Relu,
                                 bias=nbias[:], scale=isig[:])
            for s in range(8):
                tph = pp.tile([125, 128], F32, tag="tp")
                nc.tensor.transpose(tph[0:125, 0:asz],
                                    h2[:, bass.ts(s, 125)],
                                    idf[0:asz, 0:asz])
                nc.scalar.copy(h2T[:, s * CJ + a0:s * CJ + a0 + asz],
                               tph[0:125, 0:asz])

        for mi, (m0, msz) in enumerate(MT):
            jp = pp.tile([msz, CJ], F32, tag="jp", bufs=1)
            for s in range(8):
                nc.tensor.matmul(
                    jp[:], qt[:, s * MAXP + m0:s * MAXP + m0 + msz],
                    h2T[:, bass.ts(s, CJ)],
                    start=(s == 0), stop=(s == 7))
            jl = wk.tile([msz, CJ], F32, tag="jl")
            nc.scalar.activation(jl[:], jp[:], AF.Ln,
                                 bias=1.0, scale=invpt[mi][:])
            nc.vector.tensor_scalar_mul(jl[:], jl[:], 1.0 / math.log(2.0))
            nc.sync.dma_start(out[m0:m0 + msz, D:D + CJ], jl[:])

        if dbg:
            nc.sync.dma_start(d_zt[:], zs[0][:])
            nc.sync.dma_start(d_g[:], gs[0][:])
            nc.sync.dma_start(d_h2T[:], h2T[:])

    nc.compile()
    return nc


def _prep_inputs(x, atac, atac_w, joint_w, peak_split, n_peaks,
                 max_n_peaks):
    x = np.asarray(x, dtype=np.float32)
    atac = np.asarray(atac, dtype=np.float32)
    atac_w = np.asarray(atac_w, dtype=np.float32)
    joint_w = np.asarray(joint_w, dtype=np.float32)
    peak_split = np.asarray(peak_split, dtype=np.int64)
    n_peaks = np.asarray(n_peaks, dtype=np.int64)
    assert int(max_n_peaks) == MAXP and x.shape == (B, L, D)

    # x -> [B, 2, 5, 128, 25, 500] fp16 (pad D to 640)
    x16 = x.astype(np.float16)
    xpad = np.zeros((B, L, DPAD), np.float16)
    xpad[:, :, :D] = x16
    # l = (j*TCH + t)*PATCH + r ; d = dt*128 + dp
    xv = xpad.reshape(B, NJ, TCH, PATCH, NDT, 128)
    xq = np.ascontiguousarray(xv.transpose(0, 1, 4, 5, 3, 2))

    atacf = atac.reshape(B, L)

    w1 = np.ascontiguousarray(
        atac_w[:, 0, :].T / np.log(10.0)).astype(np.float32)   # [3, 161]
    w2tp = joint_w.transpose(2, 1, 0).astype(np.float16)       # [3, 800, 161]
    w2sb = np.zeros((128, 21 * CJ), np.float16)
    for dk in range(3):
        for kt in range(7):
            ksz = KSZ[kt]
            slot = dk * 7 + kt
            w2sb[0:ksz, slot * CJ:(slot + 1) * CJ] = \
                w2tp[dk, K0[kt]:K0[kt] + ksz, :]

    # ragged structure -> per-sample pool matrices and reciprocals
    q_all, inve_all, invp_all = [], [], []
    off = 0
    for c in range(B):
        npk = int(n_peaks[c])
        lens = peak_split[off:off + npk]          # elements per chunk
        off += npk + 1                            # skip trailing chunk
        plens = (lens // PATCH).astype(np.int64)  # patches per chunk
        starts = np.zeros(npk + 1, np.int64)
        starts[1:] = np.cumsum(plens)
        Q = np.zeros((LP, MAXP), np.float16)
        for i in range(npk):
            Q[starts[i]:starts[i + 1], i] = 1.0
        # [125, 8*200]: column block s holds Q rows s*125..(s+1)*125
        q_all.append(np.ascontiguousarray(
            Q.reshape(8, 125, MAXP).transpose(1, 0, 2).reshape(125, 8 * MAXP)))
        ive = np.zeros(MAXP, np.float32)
        ivp = np.zeros(MAXP, np.float32)
        ive[:npk] = 1.0 / lens.astype(np.float64)
        ivp[:npk] = 1.0 / plens.astype(np.float64)
        inve_all.append(ive)
        invp_all.append(ivp)

    in_maps = []
    for c in range(B):
        rot = np.roll(np.arange(B), -c)           # own sample -> slot 0
        atcs = atacf[rot].reshape(8, 125, 200).transpose(1, 0, 2)
        in_maps.append({
            "xq": xq[c],
            "atc": np.ascontiguousarray(atcs.reshape(125, 8 * 200)),
            "w1": w1, "w2": w2sb,
            "qsb": q_all[c],
            "inve": inve_all[c], "invp": invp_all[c],
        })
    return in_maps


def kernel(x, atac, atac_w, joint_w, peak_split, n_peaks, max_n_peaks,
           _trace=False):
    global _COMPILED
    if _COMPILED is None:
        _COMPILED = _build()
    nc = _COMPILED
    in_maps = _prep_inputs(x, atac, atac_w, joint_w, peak_split, n_peaks,
                           max_n_peaks)
    res = run_bass_kernel_spmd(nc, in_maps, list(range(NCORES)),
                               trace=_trace)
    outp = np.stack([res.results[c]["out"] for c in range(NCORES)], axis=0)
    if _trace:
        kernel._last_exec_ns = res.exec_time_ns
        kernel._last_scope_times = res.per_core_scope_times
    return outp


# revision 21
# speedup vs baseline: 1.2136x; 1.2136x over previous
"""ATACSplitPool Trainium2 kernel.

kernel(**inputs) takes the FULL inputs of the reference problem
(B=8, L=25000, D=639, PATCH=25, max_n_peaks=200) and returns the full
(8, 200, 800) float32 output, computed on 8 NeuronCores (one sample per
core, data-parallel over batch; small conv weights replicated).

Host side does only layout prep: dtype cast (fp16), padding/permutation
of x, weight transposes, and turning the ragged segment structure
(peak_split) into dense 0/1 pooling matrices + count reciprocals.
All reductions/compute over the data happen on device.

Device dataflow per core (own sample):
  - DVE fold trees over fp16 x in [128ch, 25, 500] tiles -> patch max
    (fp16) and patch sum (f32) per channel.
  - atac path replicated over the batch (atac is tiny), so BatchNorm1
    stats are local; conv1 via PE with K=3.
  - conv2 = 21 accumulating fp16 matmuls per output half (7 K-tiles x
    3 taps); BatchNorm2 stats go through a tiny AllReduce (a warm-up
    AllReduce at kernel start hides the ~70us first-collective ramp).
  - segment pooling = matmuls against host-built 0/1 matrices after
    PE transposes of the pooled data.
"""
import math
from contextlib import ExitStack

import numpy as np

import concourse.bass as bass
import concourse.tile as tile
from concourse import bacc, mybir
from concourse.bass_utils import run_bass_kernel_spmd
from concourse.masks import make_identity

dt = mybir.dt
F16 = dt.float16
F32 = dt.float32
AF = mybir.ActivationFunctionType
ALU = mybir.AluOpType

B, L, D = 8, 25000, 639
PATCH = 25
LP = 1000            # L // PATCH
MAXP = 200           # max_n_peaks
CJ = 161             # joint conv out channels
CH = 800             # conv2 in channels = D + CJ
NCORES = 8
DPAD = 640           # D padded to 5*128
NDT = 5              # D tiles of 128
TCH = 500            # fold chunk width in Lp positions
NJ = 2               # fold chunks per D tile (LP / TCH)
BN_EPS = 1e-5

KT = [128, 128, 128, 128, 127]      # x_pooled channel K tiles
K0 = [0, 128, 256, 384, 512, 639, 767]
KSZ = KT + [128, 33]
MT = [(0, 128), (128, 72)]          # chunk-row M tiles (200 rows)
AT = [(0, 128), (128, 33)]          # atac-channel tiles of 161
NT2 = [(0, 499), (499, 501)]        # conv2 t-halves (bank-sized, and the
                                    # first depends only on fold round 0)

_COMPILED = None


def _build(dbg=False, warm_cc=True):
    nc = bacc.Bacc("TRN2", target_bir_lowering=False, debug=False,
                   num_devices=NCORES)

    xq = nc.dram_tensor("xq", [NJ, NDT, 128, PATCH, TCH], F16,
                        kind="ExternalInput").ap()
    atc = nc.dram_tensor("atc", [125, 8 * 200], F32,
                         kind="ExternalInput").ap()
    w1 = nc.dram_tensor("w1", [3, CJ], F32, kind="ExternalInput").ap()
    w2 = nc.dram_tensor("w2", [128, 21 * CJ], F16,
                        kind="ExternalInput").ap()
    qsb = nc.dram_tensor("qsb", [125, 8 * MAXP], F16,
                         kind="ExternalInput").ap()
    inve = nc.dram_tensor("inve", [MAXP], F32, kind="ExternalInput").ap()
    invp = nc.dram_tensor("invp", [MAXP], F32, kind="ExternalInput").ap()
    out = nc.dram_tensor("out", [MAXP, CH], F32, kind="ExternalOutput").ap()

    cc_in = nc.dram_tensor("cc_in", [2, CJ], F32)
    cc_out = nc.dram_tensor("cc_out", [2, CJ], F32, addr_space="Shared")
    wm_in = nc.dram_tensor("wm_in", [1, 8], F32)
    wm_out = nc.dram_tensor("wm_out", [1, 8], F32, addr_space="Shared")

    if dbg:
        d_zt = nc.dram_tensor("d_zt", [128, LP], F32,
                              kind="ExternalOutput").ap()
        d_g = nc.dram_tensor("d_g", [128, 2], F32,
                             kind="ExternalOutput").ap()
        d_h2T = nc.dram_tensor("d_h2T", [125, 8 * CJ], F16,
                               kind="ExternalOutput").ap()

    with tile.TileContext(nc) as tc, ExitStack() as ctx:
        keep = ctx.enter_context(tc.tile_pool(name="keep", bufs=1))
        wk = ctx.enter_context(tc.tile_pool(name="wk", bufs=2))
        pp = ctx.enter_context(tc.tile_pool(name="pp", bufs=2, space="PSUM"))


        # ---- persistent small loads (SWDGE; keeps sync free for xq) ----
        w1t = keep.tile([3, CJ], F32)
        nc.gpsimd.dma_start(w1t[:], w1[:])
        w2t = keep.tile([128, 21 * CJ], F16)
        nc.gpsimd.dma_start(w2t[:], w2[:])
        qt = keep.tile([125, 8 * MAXP], F16)
        nc.gpsimd.dma_start(qt[:], qsb[:])
        atct = keep.tile([125, 8 * 200], F32)
        nc.scalar.dma_start(atct[:], atc[:])
        invet, invpt = [], []
        for (m0, msz) in MT:
            t_e = keep.tile([msz, 1], F32, name=f"inve{m0}")
            nc.gpsimd.dma_start(t_e[:], inve[m0:m0 + msz])
            invet.append(t_e)
            t_p = keep.tile([msz, 1], F32, name=f"invp{m0}")
            nc.gpsimd.dma_start(t_p[:], invp[m0:m0 + msz])
            invpt.append(t_p)
        idf = keep.tile([128, 128], F32)
        make_identity(nc, idf)

        # ---- warm-up collectives (hide the first-collective ramp and
        # keep the CC path warm until the real stats AllReduce) ----
        if warm_cc:
            nc.gpsimd.collective_compute(
                "AllReduce", ALU.add, replica_groups=[list(range(NCORES))],
                ins=[cc_in.ap()], outs=[cc_out.ap()])
            nc.gpsimd.collective_compute(
                "AllReduce", ALU.add, replica_groups=[list(range(NCORES))],
                ins=[wm_in.ap()], outs=[wm_out.ap()])

        # ---- persistent result buffers ----
        maxb = []
        for dtile in range(NDT):
            mb = keep.tile([128, LP + 2], F16, name=f"maxb{dtile}")
            nc.vector.memset(mb[:, 0:1], 0.0)
            nc.vector.memset(mb[:, LP + 1:LP + 2], 0.0)
            maxb.append(mb)
        S = [keep.tile([128, LP], F32, name=f"S{dtile}")
             for dtile in range(NDT)]

        # =====================================================
        # atac path (replicated over all 8 samples; own sample is in
        # slot 0 because the host rotates atc per core)
        # =====================================================
        apool = keep.tile([125, 64], F32)
        for b in range(8):
            nc.vector.reduce_max(
                apool[:, bass.ts(b, 8)],
                atct[:, bass.ts(b, 200)].rearrange("p (g r) -> p g r",
                                                   r=PATCH),
                axis=mybir.AxisListType.X)
        alog = keep.tile([125, 64], F32)
        nc.scalar.activation(alog[:], apool[:], AF.Ln, bias=1.0)

        # conv1 + BN1 stats for all samples; keep slot-0 pre-activations
        zrow = keep.tile([1, 8], F32)
        nc.vector.memset(zrow[:], 0.0)
        apre = []   # slot-0 conv1 pre-BN, zero-edged [asz, LP+2]
        for (a0, asz) in AT:
            t = keep.tile([asz, LP + 2], F32, name=f"apre{a0}")
            nc.vector.memset(t[:, 0:1], 0.0)
            nc.vector.memset(t[:, LP + 1:LP + 2], 0.0)
            apre.append(t)
        # s1p layout: cols k*16 + n*8 + b  (k: 0=sum 1=sumsq)
        s1p = [keep.tile([asz, 32], F32, name=f"s1p{a0}")
               for (a0, asz) in AT]
        for b in range(8):
            # rotating arow tile; rows: [dk, t] = A[t + dk - 1], flat DMAs
            # from alog/zrow only (chain depth 1, pipelines across samples)
            arow = wk.tile([3, LP + 2], F32, tag="arow")
            nc.scalar.dma_start(arow[0:1, 0:1], zrow[0:1, 0:1])
            nc.scalar.dma_start(arow[0:1, 1:LP + 1], alog[:, bass.ts(b, 8)])
            nc.scalar.dma_start(arow[1:2, 0:LP], alog[:, bass.ts(b, 8)])
            nc.scalar.dma_start(arow[2:3, 0:7],
                                alog[0:1, b * 8 + 1:b * 8 + 8])
            nc.scalar.dma_start(arow[2:3, 7:LP - 1],
                                alog[1:125, bass.ts(b, 8)])
            nc.scalar.dma_start(arow[2:3, LP - 1:LP], zrow[0:1, 0:1])
            for mi, (a0, asz) in enumerate(AT):
                for n in range(2):
                    za = pp.tile([asz, 512], F32, tag="pz")
                    nc.tensor.matmul(
                        za[:, 0:TCH], w1t[:, a0:a0 + asz],
                        arow[:, n * TCH:n * TCH + TCH],
                        start=True, stop=True)
                    if b == 0:
                        cdst = apre[mi][:, 1 + n * TCH:1 + (n + 1) * TCH]
                    else:
                        scr = wk.tile([asz, TCH], F16, tag=f"scr{mi}")
                        cdst = scr[:]
                    nc.scalar.activation(
                        cdst, za[:, 0:TCH], AF.Copy,
                        accum_out=s1p[mi][:, n * 8 + b:n * 8 + b + 1])
                    scr2 = wk.tile([asz, TCH], F16, tag=f"scr{mi}")
                    nc.scalar.activation(
                        scr2[:], za[:, 0:TCH], AF.Square,
                        accum_out=s1p[mi][:, 16 + n * 8 + b:17 + n * 8 + b])

        # =====================================================
        # main fold: patch max + patch sum over x (fp16, 2x DVE mode)
        # j-major so conv2's first half can start after round 0
        # =====================================================
        iop = ctx.enter_context(tc.tile_pool(name="io", bufs=2))
        fwp = ctx.enter_context(tc.tile_pool(name="fw", bufs=1))
        for j in range(NJ):
            for dtile in range(NDT):
                xin = iop.tile([128, PATCH * TCH], F16, tag="xin")
                nc.sync.dma_start(
                    xin[:], xq[j, dtile].rearrange("p r t -> p (r t)"))
                x3 = xin[:].rearrange("p (r t) -> p r t", r=PATCH)
                m12 = fwp.tile([128, 12, TCH], F16, tag="m12")
                s12 = fwp.tile([128, 12, TCH], F16, tag="s12")
                nc.vector.tensor_max(m12[:], x3[:, 0:12, :], x3[:, 12:24, :])
                nc.vector.tensor_add(s12[:], x3[:, 0:12, :], x3[:, 12:24, :])
                m6 = fwp.tile([128, 6, TCH], F16, tag="m6")
                s6 = fwp.tile([128, 6, TCH], F16, tag="s6")
                nc.vector.tensor_max(m6[:], m12[:, 0:6, :], m12[:, 6:12, :])
                nc.vector.tensor_add(s6[:], s12[:, 0:6, :], s12[:, 6:12, :])
                m3 = fwp.tile([128, 3, TCH], F16, tag="m3")
                s3 = fwp.tile([128, 3, TCH], F16, tag="s3")
                nc.vector.tensor_max(m3[:], m6[:, 0:3, :], m6[:, 3:6, :])
                nc.vector.tensor_add(s3[:], s6[:, 0:3, :], s6[:, 3:6, :])
                m1 = fwp.tile([128, 1, TCH], F16, tag="m1")
                s1 = fwp.tile([128, 1, TCH], F16, tag="s1")
                nc.vector.tensor_max(m1[:], m3[:, 0:1, :], m3[:, 1:2, :])
                nc.vector.tensor_add(s1[:], s3[:, 0:1, :], s3[:, 1:2, :])
                m1b = fwp.tile([128, 1, TCH], F16, tag="m1b")
                s1b = fwp.tile([128, 1, TCH], F16, tag="s1b")
                nc.vector.tensor_max(m1b[:], m1[:], m3[:, 2:3, :])
                nc.vector.tensor_add(s1b[:], s1[:], s3[:, 2:3, :])
                mdst = maxb[dtile][:, 1 + j * TCH:1 + j * TCH + TCH]
                nc.vector.tensor_max(
                    mdst.rearrange("p (o t) -> p o t", o=1),
                    m1b[:], x3[:, 24:25, :])
                sdst = S[dtile][:, bass.ts(j, TCH)]
                nc.vector.tensor_add(
                    sdst.rearrange("p (o t) -> p o t", o=1),
                    s1b[:], x3[:, 24:25, :])

        # BN1 coefficients (full-batch stats, local because replicated)
        def bn_coeffs(tag, asz, sums, count):
            mean = keep.tile([asz, 1], F32, name=f"m{tag}")
            nc.vector.tensor_scalar_mul(mean[:], sums[:, 0:1], 1.0 / count)
            var = keep.tile([asz, 1], F32, name=f"v{tag}")
            nc.vector.tensor_mul(var[:], mean[:], mean[:])
            msq = keep.tile([asz, 1], F32, name=f"q{tag}")
            nc.vector.tensor_scalar_mul(msq[:], sums[:, 1:2], 1.0 / count)
            nc.vector.tensor_sub(var[:], msq[:], var[:])
            nc.vector.tensor_scalar_add(var[:], var[:], BN_EPS)
            sig = keep.tile([asz, 1], F32, name=f"s{tag}")
            nc.scalar.activation(sig[:], var[:], AF.Sqrt, bias=0.0)
            isig = keep.tile([asz, 1], F32, name=f"i{tag}")
            nc.vector.reciprocal(isig[:], sig[:])
            nbias = keep.tile([asz, 1], F32, name=f"n{tag}")
            nc.vector.tensor_mul(nbias[:], mean[:], isig[:])
            nc.vector.tensor_scalar_mul(nbias[:], nbias[:], -1.0)
            return isig, nbias

        anrm = []
        for mi, (a0, asz) in enumerate(AT):
            red = keep.tile([asz, 2], F32, name=f"red1{a0}")
            nc.vector.reduce_sum(
                red[:], s1p[mi][:].rearrange("p (k b) -> p k b", b=16),
                axis=mybir.AxisListType.X)
            isig, nbias = bn_coeffs(f"1{a0}", asz, red, 8000.0)
            t = keep.tile([asz, LP + 2], F16, name=f"anrm{a0}")
            nc.vector.memset(t[:, 0:1], 0.0)
            nc.vector.memset(t[:, LP + 1:LP + 2], 0.0)
            nc.scalar.activation(t[:, 1:LP + 1], apre[mi][:, 1:LP + 1],
                                 AF.Relu, bias=nbias[:], scale=isig[:])
            anrm.append(t)

        if warm_cc:
            # keep CC warm mid-kernel: depends on S[4] round-0 fold
            nc.sync.dma_start(wm_in[0:1, 0:8], S[4][0:1, 0:8])
            nc.gpsimd.collective_compute(
                "AllReduce", ALU.add, replica_groups=[list(range(NCORES))],
                ins=[wm_in.ap()], outs=[wm_out.ap()])

        # =====================================================
        # conv2 (own sample): z = W2 * [x_pooled; a], 3 taps.
        # t-half A only needs fold round 0; half B needs round 1.
        # =====================================================
        ktiles = [maxb[0], maxb[1], maxb[2], maxb[3], maxb[4],
                  anrm[0], anrm[1]]
        zs, s2p = [], []
        for mi, (a0, asz) in enumerate(AT):
            zt = keep.tile([asz, LP], F32, name=f"zt{mi}")
            spn = keep.tile([asz, 4], F32, name=f"s2pn{mi}")
            for n, (n0, nsz) in enumerate(NT2):
                zp = pp.tile([asz, 512], F32, tag="pz")
                i = 0
                for dk in range(3):
                    for kt in range(7):
                        ksz = KSZ[kt]
                        slot = dk * 7 + kt
                        nc.tensor.matmul(
                            zp[:, 0:nsz],
                            w2t[0:ksz, slot * CJ + a0:slot * CJ + a0 + asz],
                            ktiles[kt][0:ksz, dk + n0:dk + n0 + nsz],
                            start=(i == 0), stop=(i == 20))
                        i += 1
                nc.scalar.activation(zt[:, n0:n0 + nsz], zp[:, 0:nsz],
                                     AF.Copy, accum_out=spn[:, n:n + 1])
                zsq = wk.tile([asz, 512], F16, tag=f"scr{mi}")
                nc.scalar.activation(zsq[:, 0:nsz], zp[:, 0:nsz], AF.Square,
                                     accum_out=spn[:, 2 + n:3 + n])
            sp = keep.tile([asz, 2], F32, name=f"s2p{mi}")
            nc.vector.tensor_add(
                sp[:].rearrange("p (k o) -> p k o", o=1),
                spn[:].rearrange("p (k n) -> p k n", n=2)[:, :, 0:1],
                spn[:].rearrange("p (k n) -> p k n", n=2)[:, :, 1:2])
            zs.append(zt)
            s2p.append(sp)

        # ship stats through the AllReduce
        nc.sync.dma_start(cc_in[0:1, 0:128], s2p[0][:, 0:1])
        nc.sync.dma_start(cc_in[0:1, 128:161], s2p[1][:, 0:1])
        nc.sync.dma_start(cc_in[1:2, 0:128], s2p[0][:, 1:2])
        nc.sync.dma_start(cc_in[1:2, 128:161], s2p[1][:, 1:2])
        nc.gpsimd.collective_compute(
            "AllReduce", ALU.add, replica_groups=[list(range(NCORES))],
            ins=[cc_in.ap()], outs=[cc_out.ap()])

        # =====================================================
        # x_region: transpose S, pool with Q, scale by 1/len
        # (independent of conv2/collective; overlaps fold round 1)
        # =====================================================
        sT = keep.tile([125, 8 * DPAD], F16)
        for dtile in range(NDT):
            dsz = KT[dtile]
            for s in range(8):
                tp2 = pp.tile([125, 128], F32, tag="tp")
                nc.tensor.transpose(tp2[0:125, 0:dsz],
                                    S[dtile][0:dsz, bass.ts(s, 125)],
                                    idf[0:dsz, 0:dsz])
                nc.scalar.copy(
                    sT[:, s * DPAD + dtile * 128:
                       s * DPAD + dtile * 128 + dsz],
                    tp2[0:125, 0:dsz])

        NT = [(0, 512), (512, 127)]
        for mi, (m0, msz) in enumerate(MT):
            for (n0, nsz) in NT:
                xr = pp.tile([msz, 512], F32, tag="xr", bufs=1)
                for s in range(8):
                    nc.tensor.matmul(
                        xr[:, 0:nsz],
                        qt[:, s * MAXP + m0:s * MAXP + m0 + msz],
                        sT[:, s * DPAD + n0:s * DPAD + n0 + nsz],
                        start=(s == 0), stop=(s == 7))
                xro = wk.tile([msz, 512], F32, tag="xro")
                nc.scalar.activation(xro[:, 0:nsz], xr[:, 0:nsz], AF.Copy,
                                     scale=invet[mi][:])
                nc.sync.dma_start(out[m0:m0 + msz, n0:n0 + nsz],
                                  xro[:, 0:nsz])

        # =====================================================
        # BN2 apply + relu -> h2, transpose, pool with Q, log2(mean+1)
        # =====================================================
        gs = []
        for mi, (a0, asz) in enumerate(AT):
            gsum = keep.tile([asz, 2], F32, name=f"g2{a0}")
            nc.sync.dma_start(gsum[:, 0:1], cc_out[0:1, a0:a0 + asz])
            nc.sync.dma_start(gsum[:, 1:2], cc_out[1:2, a0:a0 + asz])
            gs.append(gsum)

        h2T = keep.tile([125, 8 * CJ], F16)
        for mi, (a0, asz) in enumerate(AT):
            isig, nbias = bn_coeffs(f"2{a0}", asz, gs[mi], 8000.0)
            h2 = keep.tile([asz, LP], F32, name=f"h2{mi}")
            nc.scalar.activation(h2[:], zs[mi][:], AF.Relu,
                                 bias=nbias[:], scale=isig[:])
            for s in range(8):
                tph = pp.tile([125, 128], F32, tag="tp")
                nc.tensor.transpose(tph[0:125, 0:asz],
                                    h2[:, bass.ts(s, 125)],
                                    idf[0:asz, 0:asz])
                nc.scalar.copy(h2T[:, s * CJ + a0:s * CJ + a0 + asz],
                               tph[0:125, 0:asz])

        for mi, (m0, msz) in enumerate(MT):
            jp = pp.tile([msz, CJ], F32, tag="jp", bufs=1)
            for s in range(8):
                nc.tensor.matmul(
                    jp[:], qt[:, s * MAXP + m0:s * MAXP + m0 + msz],
                    h2T[:, bass.ts(s, CJ)],
                    start=(s == 0), stop=(s == 7))
            jl = wk.tile([msz, CJ], F32, tag="jl")
            nc.scalar.activation(jl[:], jp[:], AF.Ln,
                                 bias=1.0, scale=invpt[mi][:])
            nc.vector.tensor_scalar_mul(jl[:], jl[:], 1.0 / math.log(2.0))
            nc.sync.dma_start(out[m0:m0 + msz, D:D + CJ], jl[:])

        if dbg:
            nc.sync.dma_start(d_zt[:], zs[0][:])
            nc.sync.dma_start(d_g[:], gs[0][:])
            nc.sync.dma_start(d_h2T[:], h2T[:])

    nc.compile()
    return nc


def _prep_inputs(x, atac, atac_w, joint_w, peak_split, n_peaks,
                 max_n_peaks):
    x = np.asarray(x, dtype=np.float32)
    atac = np.asarray(atac, dtype=np.float32)
    atac_w = np.asarray(atac_w, dtype=np.float32)
    joint_w = np.asarray(joint_w, dtype=np.float32)
    peak_split = np.asarray(peak_split, dtype=np.int64)
    n_peaks = np.asarray(n_peaks, dtype=np.int64)
    assert int(max_n_peaks) == MAXP and x.shape == (B, L, D)

    # x -> [B, 2, 5, 128, 25, 500] fp16 (pad D to 640)
    x16 = x.astype(np.float16)
    xpad = np.zeros((B, L, DPAD), np.float16)
    xpad[:, :, :D] = x16
    # l = (j*TCH + t)*PATCH + r ; d = dt*128 + dp
    xv = xpad.reshape(B, NJ, TCH, PATCH, NDT, 128)
    xq = np.ascontiguousarray(xv.transpose(0, 1, 4, 5, 3, 2))

    atacf = atac.reshape(B, L)

    w1 = np.ascontiguousarray(
        atac_w[:, 0, :].T / np.log(10.0)).astype(np.float32)   # [3, 161]
    w2tp = joint_w.transpose(2, 1, 0).astype(np.float16)       # [3, 800, 161]
    w2sb = np.zeros((128, 21 * CJ), np.float16)
    for dk in range(3):
        for kt in range(7):
            ksz = KSZ[kt]
            slot = dk * 7 + kt
            w2sb[0:ksz, slot * CJ:(slot + 1) * CJ] = \
                w2tp[dk, K0[kt]:K0[kt] + ksz, :]

    # ragged structure -> per-sample pool matrices and reciprocals
    q_all, inve_all, invp_all = [], [], []
    off = 0
    for c in range(B):
        npk = int(n_peaks[c])
        lens = peak_split[off:off + npk]          # elements per chunk
        off += npk + 1                            # skip trailing chunk
        plens = (lens // PATCH).astype(np.int64)  # patches per chunk
        starts = np.zeros(npk + 1, np.int64)
        starts[1:] = np.cumsum(plens)
        Q = np.zeros((LP, MAXP), np.float16)
        for i in range(npk):
            Q[starts[i]:starts[i + 1], i] = 1.0
        # [125, 8*200]: column block s holds Q rows s*125..(s+1)*125
        q_all.append(np.ascontiguousarray(
            Q.reshape(8, 125, MAXP).transpose(1, 0, 2).reshape(125, 8 * MAXP)))
        ive = np.zeros(MAXP, np.float32)
        ivp = np.zeros(MAXP, np.float32)
        ive[:npk] = 1.0 / lens.astype(np.float64)
        ivp[:npk] = 1.0 / plens.astype(np.float64)
        inve_all.append(ive)
        invp_all.append(ivp)

    in_maps = []
    for c in range(B):
        rot = np.roll(np.arange(B), -c)           # own sample -> slot 0
        atcs = atacf[rot].reshape(8, 125, 200).transpose(1, 0, 2)
        in_maps.append({
            "xq": xq[c],
            "atc": np.ascontiguousarray(atcs.reshape(125, 8 * 200)),
            "w1": w1, "w2": w2sb,
            "qsb": q_all[c],
            "inve": inve_all[c], "invp": invp_all[c],
        })
    return in_maps


def kernel(x, atac, atac_w, joint_w, peak_split, n_peaks, max_n_peaks,
           _trace=False):
    global _COMPILED
    if _COMPILED is None:
        _COMPILED = _build()
    nc = _COMPILED
    in_maps = _prep_inputs(x, atac, atac_w, joint_w, peak_split, n_peaks,
                           max_n_peaks)
    res = run_bass_kernel_spmd(nc, in_maps, list(range(NCORES)),
                               trace=_trace)
    outp = np.stack([res.results[c]["out"] for c in range(NCORES)], axis=0)
    if _trace:
        kernel._last_exec_ns = res.exec_time_ns
        kernel._last_scope_times = res.per_core_scope_times
    return outp
